# revision 1
# baseline (speedup 1.0000x reference)
"""DiscreteWaveletUpsample Trainium2 kernel.

Math: out = conv3x3(haar_upsample(conv3x3(x, pre_w) + pre_b), post_w) + post_b

Device algorithm (per core, one batch sample, data-parallel over batch=8):

  * The fixed Haar reconstruction (stride-2 transposed conv with
    non-overlapping 2x2 taps) is folded into the pre-conv weights:
    Y(p,q)[c,h,w] (the (p,q) polyphase components of the upsampled image,
    y[c, 2h+p, 2w+q] = Y(p,q)[c,h,w]) is itself a 3x3 conv of x with
    effective weights  Weff[p,q,c] = sum_s haar[s,p,q] * pre_w[s*64+c].

  * Stage 1 (per 4-row spatial tile, out ctile p): 9 tap-matmuls with
    K=cin=64 accumulate [Y(p,0); Y(p,1)] (M=128) in PSUM.  The PE runs in
    64x128 row-tiled mode: x is duplicated on both partition halves and
    tiles alternate row groups by spatial parity, so two tiles' matmul
    streams execute concurrently on the two sub-arrays (K=64 serial
    matmuls would otherwise run at half rate and never warm the HAM
    clock gate).  Evacuation (ScalarE/VectorE alternating) adds the bias
    and writes bf16 into zero-padded SBUF images.

  * The Y images are duplicated with partition halves swapped (SBUF to
    SBUF DMA) so that every polyphase component is available on both
    partition halves.

  * Stage 2 = the post conv in polyphase space: output component (p,q)
    at (h,w) sums 9 taps, each a K=64 matmul against component
    (p_in,q_in) at offset (dy,dx) (from the polyphase decomposition of
    the 3x3 kernel).  The PE runs in 64x64 four-tile mode: row group =
    component parity (q), column group = spatial-tile parity, so four
    matmul streams execute concurrently (full-array throughput at
    K=64/M=64, ~59 ns per 512-col matmul measured vs 223 serial).
    Evacuation adds post_b and interleaves components into
    full-resolution rows in SBUF staging; col-group-1 tiles land on
    partitions 64-127 and DMA out from there.

  * Full-res rows are DMA'd to HBM contiguously (512 KB per tile).
"""

import os

import numpy as np
import ml_dtypes

import concourse.bass as bass
import concourse.mybir as mybir
import concourse.tile as tile
from concourse import bacc
from concourse.tile_rust import add_dep_helper
from concourse.bass_utils import run_bass_kernel_spmd

N_CORES = 8

# The bass compile pipeline pins --enable-ldw-opt=false; this kernel's
# matmul streams reload identical weights constantly (conv taps), and
# walrus's ldw-opt pass elides those reloads with correct weight-buffer
# bookkeeping.  Rewrite the flag on the walrus command line.
if os.environ.get("KERNEL_LDW_OPT", "0") == "1":
    import concourse.bass_utils as _bu

    if not getattr(_bu, "_ldw_opt_patched", False):
        _orig_run_command = _bu.run_command

        def _run_command_ldw_opt(argv, **kwargs):
            argv = ["--enable-ldw-opt=true" if a == "--enable-ldw-opt=false"
                    else a for a in argv]
            return _orig_run_command(argv, **kwargs)

        _bu.run_command = _run_command_ldw_opt
        _bu._ldw_opt_patched = True

C = 64          # channels (cin = cout = 64; stage-1 produces 4*C subbands)
H = W = 128     # input spatial dims
HP, WP = H + 2, W + 2   # zero-padded
TAPS9 = [(ky, kx) for ky in range(3) for kx in range(3)]
COMPS = [(0, 0), (0, 1), (1, 0), (1, 1)]

F32 = mybir.dt.float32
BF16 = mybir.dt.bfloat16
NP_BF16 = ml_dtypes.bfloat16

IDENT = mybir.ActivationFunctionType.Identity


# ----------------------------------------------------------------------------
# Host-side weight preparation
# ----------------------------------------------------------------------------

def _build_stage1_weights(pre_w, pre_b):
    """Fold the Haar reconstruction into the pre-conv weights.

    Returns
      w1[p, ky, kx, cin, m] float32, m = q*64 + c
      b1[m, p] float32
    """
    lo = np.array([0.5, 0.5], np.float32)
    hi = np.array([0.5, -0.5], np.float32)
    filt = np.stack([np.outer(lo, lo), np.outer(lo, hi),
                     np.outer(hi, lo), np.outer(hi, hi)], axis=0)  # [4,2,2]
    pw = pre_w.reshape(4, C, C, 3, 3).astype(np.float32)
    pb = pre_b.reshape(4, C).astype(np.float32)
    weff = np.einsum('spq,scikl->pqcikl', filt, pw)   # [p,q,c,cin,ky,kx]
    beff = np.einsum('spq,sc->pqc', filt, pb)         # [p,q,c]
    w1 = np.transpose(weff, (0, 4, 5, 3, 1, 2)).reshape(2, 3, 3, C, 2 * C)
    b1 = beff.reshape(2, 2 * C).T.copy()              # [m, p]
    return w1, b1


def _tap_decomp(p, q, ky, kx):
    """Polyphase decomposition of full-res tap (ky,kx) for out comp (p,q):
    returns (p_in, q_in, dy, dx)."""
    jy = p + ky - 1
    p_in = jy & 1
    dy = (jy - p_in) >> 1
    jx = q + kx - 1
    q_in = jx & 1
    dx = (jx - q_in) >> 1
    return p_in, q_in, dy, dx


def _build_stage2_weights(post_w):
    """w2[128, 9*64] bf16-ready float32.

    The stage-2 lhsT depends only on the tap: column block ti holds
    post_w[:, :, ky, kx].T [cin 64, cout 64], duplicated on both
    partition halves so either row group can load it."""
    w2 = np.zeros((2 * C, 9 * C), np.float32)
    pwf = post_w.astype(np.float32)
    for ti, (ky, kx) in enumerate(TAPS9):
        blk = ti * C
        w2[0:C, blk:blk + C] = pwf[:, :, ky, kx].T
        w2[C:2 * C, blk:blk + C] = pwf[:, :, ky, kx].T
    return w2


# ----------------------------------------------------------------------------
# Device module
# ----------------------------------------------------------------------------

PARTS = os.environ.get("KERNEL_PARTS", "all")  # debug bisection: s1 / s2 / all


def _build_module():
    nc = bacc.Bacc("TRN2", target_bir_lowering=False, debug=False,
                   num_devices=N_CORES)

    x_d = nc.dram_tensor("x", [C, H, W], BF16, kind="ExternalInput")
    w1_d = nc.dram_tensor("w1", [128, 18 * 128], BF16, kind="ExternalInput")
    b1_d = nc.dram_tensor("b1", [128, 2], F32, kind="ExternalInput")
    w2_d = nc.dram_tensor("w2", [128, 9 * C], BF16, kind="ExternalInput")
    b2_d = nc.dram_tensor("b2", [128, 1], F32, kind="ExternalInput")
    out_d = nc.dram_tensor("out", [C, 2 * H, 2 * W], F32,
                           kind="ExternalOutput")

    with tile.TileContext(nc) as tc:
        with (
            tc.tile_pool(name="const", bufs=1) as const,
            tc.tile_pool(name="xbuf", bufs=1) as xpool,
            tc.tile_pool(name="ybuf", bufs=1) as ypool,
            tc.tile_pool(name="psum", bufs=8, space="PSUM") as psum_pool,
            tc.tile_pool(name="stage", bufs=4) as stg,
        ):
            # ---- constants ----
            w1_s = const.tile([128, 18 * 128], BF16)
            nc.sync.dma_start(out=w1_s[:], in_=w1_d[:])
            w2_s = const.tile([128, 9 * C], BF16)
            nc.sync.dma_start(out=w2_s[:], in_=w2_d[:])
            b1_s = const.tile([128, 2], F32)
            nc.sync.dma_start(out=b1_s[:], in_=b1_d[:])
            b2_s = const.tile([128, 1], F32)
            nc.sync.dma_start(out=b2_s[:], in_=b2_d[:])

            # ---- input image, zero-padded, duplicated on both halves ----
            # Border-only memsets: disjoint from the DMA'd interior, so the
            # first matmuls don't wait on a whole-tile clear (a full-tile
            # GpSimd memset put ~16us on the critical path).
            xp = xpool.tile([128, HP, WP], BF16)
            nc.vector.memset(xp[:, 0, :], 0.0)
            nc.vector.memset(xp[:, HP - 1, :], 0.0)
            nc.vector.memset(xp[:, 1:H + 1, 0:1], 0.0)
            nc.vector.memset(xp[:, 1:H + 1, WP - 1:WP], 0.0)
            # Chunked load: the strided padded-layout writes are slow
            # (256 B runs), and the in-order PE queue stalls on the first
            # g=1 matmul until the second duplicate half lands.  Row-block
            # chunks let super s start once its rows arrive.
            # ... and the two duplicate halves ride separate DMA queues.
            for r0 in range(0, H, 32):
                for g, eng in ((0, nc.sync), (1, nc.gpsimd)):
                    eng.dma_start(
                        out=xp[g * C:(g + 1) * C, 1 + r0:1 + r0 + 32,
                               1:W + 1],
                        in_=x_d[:, r0:r0 + 32, :])

            # ---- Y buffers ----
            # ybufs[p][j]: partitions 0-63 = Y(p,j), 64-127 = Y(p,1-j);
            # j=0 written by stage-1 evac, j=1 is the partition-swapped DMA
            # copy.  Comp (p_in,q_in) on half g lives in ybufs[p_in][q_in^g].
            ybufs = [[None, None], [None, None]]
            for p in (0, 1):
                for j in (0, 1):
                    yb = ypool.tile([128, HP, WP], BF16, name=f"ybuf{p}{j}")
                    ybufs[p][j] = yb
                    if PARTS == "s2":
                        nc.gpsimd.memset(yb[:], 0.0)
                        continue
                    nc.gpsimd.memset(yb[:, 0, :], 0.0)
                    nc.gpsimd.memset(yb[:, HP - 1, :], 0.0)
                    if j == 0:
                        # interior written by evac; dup copies full width
                        nc.gpsimd.memset(yb[:, :, 0], 0.0)
                        nc.gpsimd.memset(yb[:, :, WP - 1], 0.0)

            # ---- matmul emission: global PE order chain (the emission
            # order interleaves sub-arrays for concurrency; pinning it also
            # makes the post-compile LDWEIGHTS dedup sound) ----
            state = {"prev": None}

            def mm(out_ap, w_ap, w_key, rhs_ap, start, stop, pos):
                inst = nc.tensor.matmul(out_ap, w_ap, rhs_ap,
                                        start=start, stop=stop,
                                        tile_position=pos)
                if state["prev"] is not None:
                    add_dep_helper(inst.ins, state["prev"], sync=False,
                                   reason="pe-emission-order")
                state["prev"] = inst.ins

            def stage1_super(sup):
                # Supers of 8 spatial tiles, split by out-ctile p: the 8 PSUM
                # banks hold one p-phase; taps outermost so each tap's
                # weights load once per sub-array, reused for 4 matmuls.
                # Uniform 64x64 four-tile mode: row group g = tile parity,
                # col group cg = the q-half of the 128 output channels.
                ts_all = list(range(4 * sup, 4 * sup + 4))
                for p in (0, 1):
                    accs = {}
                    for t in ts_all:
                        accs[t] = psum_pool.tile([128, 4, W], F32,
                                                 name="ps", tag="ps")
                    for k, (ky, kx) in enumerate(TAPS9):
                        idx = (ky * 3 + kx) * 2 + p
                        for g in (0, 1):
                            gs = slice(g * C, (g + 1) * C)
                            for t in ts_all[g::2]:
                                h0 = 4 * t
                                mm(accs[t][:, :, :],
                                   w1_s[gs, idx * 128:(idx + 1) * 128],
                                   ("s1", idx),
                                   xp[gs, h0 + ky:h0 + ky + 4, kx:kx + W],
                                   k == 0, k == 8, (g * C, 0))
                    for t in ts_all:
                        h0 = 4 * t
                        dst = ybufs[p][0][:, h0 + 1:h0 + 5, 1:W + 1]
                        if t % 2 == 0:
                            nc.scalar.activation(dst, accs[t][:, :, :], IDENT,
                                                 bias=b1_s[:, p:p + 1])
                        else:
                            nc.vector.tensor_scalar_add(dst, accs[t][:, :, :],
                                                        b1_s[:, p:p + 1])
                        # duplicate with partition halves swapped
                        nc.sync.dma_start(
                            out=ybufs[p][1][0:C, h0 + 1:h0 + 5, :],
                            in_=ybufs[p][0][C:128, h0 + 1:h0 + 5, :])
                        nc.sync.dma_start(
                            out=ybufs[p][1][C:128, h0 + 1:h0 + 5, :],
                            in_=ybufs[p][0][0:C, h0 + 1:h0 + 5, :])

            def stage2_block(j):
                # Four-tile mode: row group g = comp q, col group c = spatial
                # parity.  Per-tile PSUM granularity: each spatial tile
                # allocates its 4 accumulators on entry, so bank releases
                # pipeline at half-j granularity.  The two p-matmuls per
                # (tap, sub-array) share the tap's weights.
                st = stg.tile([128, 8, 2 * W], F32, name="st", tag="st")
                for c in (0, 1):
                    t = 2 * j + c
                    h0 = 4 * t
                    cs = slice(c * C, (c + 1) * C)
                    accs = {}
                    for p, q in COMPS:
                        accs[p, q] = psum_pool.tile([128, 4, W], F32,
                                                    name="ps", tag="ps")
                    for i, (ky, kx) in enumerate(TAPS9):
                        for q in (0, 1):
                            g = q
                            gs = slice(g * C, (g + 1) * C)
                            blk = i * C
                            for p in (0, 1):
                                p_in, q_in, dy, dx = _tap_decomp(p, q, ky, kx)
                                rhs = ybufs[p_in][q_in ^ g][
                                    gs, h0 + dy + 1:h0 + dy + 5,
                                    1 + dx:1 + dx + W]
                                mm(accs[p, q][cs, :, :],
                                   w2_s[gs, blk:blk + C],
                                   ("s2", ky, kx),
                                   rhs, i == 0, i == 8, (g * C, c * C))
                    for p, q in COMPS:
                        dst = st[cs, p::2, q::2]
                        if (p * 2 + q + c) % 2 == 0:
                            nc.scalar.activation(dst, accs[p, q][cs, :, :],
                                                 IDENT, bias=b2_s[cs, 0:1])
                        else:
                            nc.vector.tensor_scalar_add(
                                dst, accs[p, q][cs, :, :], b2_s[cs, 0:1])
                    nc.sync.dma_start(out=out_d[:, 8 * t:8 * t + 8, :],
                                      in_=st[cs, :, :])

            # ---- interleaved emission: stage-2 block j needs stage-1 tiles
            # through 2j+2, i.e. supers through ceil((2j+2-7)/8) ----
            n_sup = H // 16
            if PARTS == "s1":
                for sup in range(n_sup):
                    stage1_super(sup)
                nc.gpsimd.dma_start(out=out_d[:, 0:128, 0:130],
                                    in_=ybufs[0][0][0:C, 0:128, :])
                nc.gpsimd.dma_start(out=out_d[:, 128:256, 0:130],
                                    in_=ybufs[1][0][0:C, 0:128, :])
            elif PARTS == "s2":
                for j in range(H // 8):
                    stage2_block(j)
            elif os.environ.get("KERNEL_SEQ", "0") == "1":
                # Sequential stages: exactly one PE tiling-mode switch,
                # which the LDWEIGHTS dedup pass tolerates.
                for sup in range(n_sup):
                    stage1_super(sup)
                for j in range(H // 8):
                    stage2_block(j)
            else:
                next_j = 0
                for sup in range(n_sup):
                    stage1_super(sup)
                    j_hi = min(2 * sup, H // 8 - 1)
                    if sup == n_sup - 1:
                        j_hi = H // 8 - 1
                    while next_j <= j_hi:
                        stage2_block(next_j)
                        next_j += 1

    if os.environ.get("KERNEL_LDW_OPT", "0") == "1":
        nc.move_matmul_waits_to_ldweights = lambda: None
    nc.compile()
    if os.environ.get("KERNEL_DEDUP_LDW", "0") == "1":
        removed = _dedup_ldweights(nc)
        _verify_weight_state(nc)
    return nc


def _verify_weight_state(nc):
    """Check every matmul's stationary operand is what its sub-array last
    loaded (post-dedup logical soundness)."""
    for f in nc.m.functions:
        for bb in f.blocks:
            loaded = {}
            for ins in bb.instructions:
                if isinstance(ins, mybir.InstLdweights):
                    loaded[str(ins.tile_position)] = _ldw_sig(ins)
                elif isinstance(ins, mybir.InstMatmult):
                    pap = ins.ins[1]
                    sig = (str(getattr(pap, "memref", "")),
                           str(getattr(pap, "offset", "")), str(pap.ap),
                           str(ins.tile_position), str(ins.tile_size))
                    assert loaded.get(str(ins.tile_position)) == sig, (
                        ins.name, sig, loaded.get(str(ins.tile_position)))
                elif type(ins).__name__ in ("InstDrain", "InstISA"):
                    loaded.clear()
    return True


def _ldw_sig(ins):
    pap = ins.ins[0]
    return (str(getattr(pap, "memref", "")), str(getattr(pap, "offset", "")),
            str(pap.ap), str(ins.tile_position), str(ins.tile_size))


def _dedup_ldweights(nc):
    """Drop InstLdweights whose sub-array already holds the same weights.

    Bacc's compile pass splits every self-loading matmul into a standalone
    InstLdweights + non-self-loading InstMatmult pair.  Consecutive matmuls
    on the same PE sub-array with identical stationary operands reload
    identical weights; the reloads are pure overhead (LDWEIGHTS runs fully
    serialized with matmuls on TRN2, ~50 ns each).  Per-sub-array emission
    order was pinned with nosync deps at build time, so the instruction
    order here is the execution order and the elision is sound.  Any other
    PE instruction (drain, mode switch implied by different tile_size)
    invalidates the tracked state."""
    removed = 0
    for f in nc.m.functions:
        for bb in f.blocks:
            loaded = {}
            last_tile_size = None
            new_list = []
            for ins in bb.instructions:
                if isinstance(ins, mybir.InstLdweights):
                    ts = str(ins.tile_size)
                    if last_tile_size is not None and ts != last_tile_size:
                        loaded.clear()   # PE mode switch reconfigures array
                    last_tile_size = ts
                    sig = _ldw_sig(ins)
                    pos = str(ins.tile_position)
                    si = ins.sync_info
                    has_sync = si is not None and (si.on_wait or si.on_update)
                    if loaded.get(pos) == sig and not has_sync:
                        removed += 1
                        continue
                    loaded[pos] = sig
                    new_list.append(ins)
                elif isinstance(ins, mybir.InstMatmult):
                    new_list.append(ins)
                else:
                    if type(ins).__name__ in ("InstDrain", "InstISA"):
                        loaded.clear()
                    new_list.append(ins)
            del bb.instructions[:]
            for ins in new_list:
                bb.instructions.append(ins)
    return removed


_MODULE_CACHE = {}


def _get_module():
    if "nc" not in _MODULE_CACHE:
        _MODULE_CACHE["nc"] = _build_module()
    return _MODULE_CACHE["nc"]


# ----------------------------------------------------------------------------
# Entry point
# ----------------------------------------------------------------------------

def prep_weight_map(pre_w, pre_b, post_w, post_b):
    """Device-layout weight arrays, shared across cores."""
    w1, b1 = _build_stage1_weights(np.asarray(pre_w), np.asarray(pre_b))
    w2 = _build_stage2_weights(np.asarray(post_w))
    b2 = np.asarray(post_b, np.float32).reshape(C, 1)

    w1_half = np.transpose(w1, (3, 1, 2, 0, 4)).reshape(C, 18 * 128)
    # w1_half[cin, ((ky*3+kx)*2+p)*128 + m] = w1[p, ky, kx, cin, m]
    w1_flat = np.ascontiguousarray(
        np.concatenate([w1_half, w1_half], axis=0)).astype(NP_BF16)
    w2_flat = np.ascontiguousarray(w2).astype(NP_BF16)
    return {
        "w1": w1_flat,
        "b1": np.ascontiguousarray(b1, np.float32),                # [128, 2]
        "w2": w2_flat,
        "b2": np.ascontiguousarray(np.vstack([b2, b2]), np.float32),
    }


def run(x, pre_w, pre_b, post_w, post_b, trace=False):
    x = np.asarray(x, np.float32)
    B = x.shape[0]
    assert B == N_CORES and x.shape == (B, C, H, W)

    wmap = prep_weight_map(pre_w, pre_b, post_w, post_b)
    x_bf = x.astype(NP_BF16)

    in_maps = []
    for b in range(B):
        in_maps.append({"x": np.ascontiguousarray(x_bf[b]), **wmap})

    nc = _get_module()
    res = run_bass_kernel_spmd(nc, in_maps, core_ids=list(range(N_CORES)),
                               trace=trace)
    out = np.stack([res.results[b]["out"] for b in range(B)])
    return out, res


def kernel(x, pre_w, pre_b, post_w, post_b):
    out, _ = run(x, pre_w, pre_b, post_w, post_b)
    return out



# revision 4
# speedup vs baseline: 1.3413x; 1.3413x over previous
"""DiscreteWaveletUpsample Trainium2 kernel.

Math: out = conv3x3(haar_upsample(conv3x3(x, pre_w) + pre_b), post_w) + post_b

Device algorithm (per core, one batch sample, data-parallel over batch=8):

  * The fixed Haar reconstruction (stride-2 transposed conv with
    non-overlapping 2x2 taps) is folded into the pre-conv weights:
    Y(p,q)[c,h,w] (the (p,q) polyphase components of the upsampled image,
    y[c, 2h+p, 2w+q] = Y(p,q)[c,h,w]) is itself a 3x3 conv of x with
    effective weights  Weff[p,q,c] = sum_s haar[s,p,q] * pre_w[s*64+c].

  * All SBUF images are DENSE (no zero-pad halo).  Border taps emit
    narrower matmuls into row/col-offset PSUM sub-windows; the always-
    interior tap (ky=1,kx=1) goes first so its start-flag clears the
    whole accumulator.  Dense layout keeps every DMA run >= 4 KB
    (the padded layout's 256 B runs ran the input load at 62 GB/s).

  * Stage 1 (mini-super = 2 spatial 4-row tiles, per out ctile p): 9
    tap-matmuls with K=cin=64 accumulate [Y(p,0); Y(p,1)] (M=128) in
    PSUM.  The PE runs in 64x128 row-tiled mode: x is duplicated on both
    partition halves and the two tiles alternate row groups, so both
    matmul streams execute concurrently on the two sub-arrays.
    Evacuation (ScalarE/VectorE alternating) adds the bias and writes
    bf16 into dense SBUF images.

  * The Y images are duplicated with partition halves swapped (SBUF to
    SBUF DMA on the Scalar/Vector queues) so every polyphase component
    is available on both partition halves.

  * Stage 2 = the post conv in polyphase space: output component (p,q)
    at (h,w) sums 9 taps, each a K=64 matmul against component
    (p_in,q_in) at offset (dy,dx).  The PE runs in 64x64 four-tile
    mode: row group = component parity (q), column group = spatial-tile
    parity.  The four accumulators are SHARED between the two spatial
    tiles of a block (col group c writes PSUM partitions [64c, 64c+64)),
    so a block needs only 4 PSUM banks and blocks double-buffer.
    Emission round-robins single matmuls across the four (q, c) streams
    so the in-order PE queue keeps all four sub-arrays streaming (the
    old tile-major emission capped concurrency at ~2.2 of 4).
    Evacuation is full-width (both col groups at once) and interleaves
    components into full-resolution rows in SBUF staging.

  * Full-res rows DMA to HBM per block, alternating Sync/GpSimd queues;
    the x load rides Sync+GpSimd up front and weights ride Scalar, so
    no queue serializes compute-critical transfers behind bulk output.
"""

import os

import numpy as np
import ml_dtypes

import concourse.bass as bass
import concourse.mybir as mybir
import concourse.tile as tile
from concourse import bacc
from concourse.tile_rust import add_dep_helper
from concourse.bass_utils import run_bass_kernel_spmd

N_CORES = 8

C = 64          # channels (cin = cout = 64; stage-1 produces 4*C subbands)
H = W = 128     # input spatial dims
TAPS9 = [(ky, kx) for ky in range(3) for kx in range(3)]
# interior-full tap first: its start-flag write covers the whole acc
TAP_ORDER = [(1, 1)] + [t for t in TAPS9 if t != (1, 1)]
COMPS = [(0, 0), (0, 1), (1, 0), (1, 1)]

F32 = mybir.dt.float32
BF16 = mybir.dt.bfloat16
NP_BF16 = ml_dtypes.bfloat16

IDENT = mybir.ActivationFunctionType.Identity


# ----------------------------------------------------------------------------
# Host-side weight preparation
# ----------------------------------------------------------------------------

def _build_stage1_weights(pre_w, pre_b):
    """Fold the Haar reconstruction into the pre-conv weights.

    Returns
      w1[p, ky, kx, cin, m] float32, m = q*64 + c
      b1[m, p] float32
    """
    lo = np.array([0.5, 0.5], np.float32)
    hi = np.array([0.5, -0.5], np.float32)
    filt = np.stack([np.outer(lo, lo), np.outer(lo, hi),
                     np.outer(hi, lo), np.outer(hi, hi)], axis=0)  # [4,2,2]
    pw = pre_w.reshape(4, C, C, 3, 3).astype(np.float32)
    pb = pre_b.reshape(4, C).astype(np.float32)
    weff = np.einsum('spq,scikl->pqcikl', filt, pw)   # [p,q,c,cin,ky,kx]
    beff = np.einsum('spq,sc->pqc', filt, pb)         # [p,q,c]
    w1 = np.transpose(weff, (0, 4, 5, 3, 1, 2)).reshape(2, 3, 3, C, 2 * C)
    b1 = beff.reshape(2, 2 * C).T.copy()              # [m, p]
    return w1, b1


def _tap_decomp(p, q, ky, kx):
    """Polyphase decomposition of full-res tap (ky,kx) for out comp (p,q):
    returns (p_in, q_in, dy, dx)."""
    jy = p + ky - 1
    p_in = jy & 1
    dy = (jy - p_in) >> 1
    jx = q + kx - 1
    q_in = jx & 1
    dx = (jx - q_in) >> 1
    return p_in, q_in, dy, dx


def _build_stage2_weights(post_w):
    """w2[128, 9*64] bf16-ready float32.

    The stage-2 lhsT depends only on the tap: column block ti holds
    post_w[:, :, ky, kx].T [cin 64, cout 64], duplicated on both
    partition halves so either row group can load it."""
    w2 = np.zeros((2 * C, 9 * C), np.float32)
    pwf = post_w.astype(np.float32)
    for ti, (ky, kx) in enumerate(TAPS9):
        blk = ti * C
        w2[0:C, blk:blk + C] = pwf[:, :, ky, kx].T
        w2[C:2 * C, blk:blk + C] = pwf[:, :, ky, kx].T
    return w2


# ----------------------------------------------------------------------------
# Device module
# ----------------------------------------------------------------------------

def _build_module():
    nc = bacc.Bacc("TRN2", target_bir_lowering=False, debug=False,
                   num_devices=N_CORES)

    x_d = nc.dram_tensor("x", [C, H, W], BF16, kind="ExternalInput")
    w1_d = nc.dram_tensor("w1", [128, 18 * 128], BF16, kind="ExternalInput")
    b1_d = nc.dram_tensor("b1", [128, 2], F32, kind="ExternalInput")
    w2_d = nc.dram_tensor("w2", [128, 9 * C], BF16, kind="ExternalInput")
    b2_d = nc.dram_tensor("b2", [128, 1], F32, kind="ExternalInput")
    out_d = nc.dram_tensor("out", [C, 2 * H, 2 * W], F32,
                           kind="ExternalOutput")

    with tile.TileContext(nc) as tc:
        with (
            tc.tile_pool(name="const", bufs=1) as const,
            tc.tile_pool(name="xbuf", bufs=1) as xpool,
            tc.tile_pool(name="ybuf", bufs=1) as ypool,
            tc.tile_pool(name="psum", bufs=8, space="PSUM") as psum_pool,
            tc.tile_pool(name="stage", bufs=4) as stg,
        ):
            # ---- constants: Scalar queue, off the x-load queues ----
            w1_s = const.tile([128, 18 * 128], BF16)
            nc.scalar.dma_start(out=w1_s[:], in_=w1_d[:])
            w2_s = const.tile([128, 9 * C], BF16)
            nc.scalar.dma_start(out=w2_s[:], in_=w2_d[:])
            b1_s = const.tile([128, 2], F32)
            nc.scalar.dma_start(out=b1_s[:], in_=b1_d[:])
            b2_s = const.tile([128, 1], F32)
            nc.scalar.dma_start(out=b2_s[:], in_=b2_d[:])

            # ---- input image, dense, duplicated on both halves ----
            # 32-row chunks, halves on separate queues: the first
            # mini-super starts once chunk 0 of both halves lands.
            x_s = xpool.tile([128, H, W], BF16)
            for r0 in range(0, H, 32):
                nc.sync.dma_start(out=x_s[0:C, r0:r0 + 32, :],
                                  in_=x_d[:, r0:r0 + 32, :])
                nc.gpsimd.dma_start(out=x_s[C:128, r0:r0 + 32, :],
                                    in_=x_d[:, r0:r0 + 32, :])

            # ---- Y buffers, dense ----
            # ybufs[p][j]: partitions 0-63 = Y(p,j), 64-127 = Y(p,1-j);
            # j=0 written by stage-1 evac, j=1 is the partition-swapped
            # DMA copy.  Comp (p_in,q_in) on half g is ybufs[p_in][q_in^g].
            # Every element is evac-written before stage-2 reads it, so
            # no memsets are needed.
            ybufs = [[None, None], [None, None]]
            for p in (0, 1):
                for j in (0, 1):
                    ybufs[p][j] = ypool.tile([128, H, W], BF16,
                                             name=f"ybuf{p}{j}")

            # ---- matmul emission: global PE order chain ----
            state = {"prev": None}

            def mm(out_ap, w_ap, rhs_ap, start, stop, pos):
                inst = nc.tensor.matmul(out_ap, w_ap, rhs_ap,
                                        start=start, stop=stop,
                                        tile_position=pos)
                if state["prev"] is not None:
                    add_dep_helper(inst.ins, state["prev"], sync=False,
                                   reason="pe-emission-order")
                state["prev"] = inst.ins

            def stage1_ms(ms):
                # Mini-super of 2 spatial tiles: 4 PSUM banks, so
                # mini-supers and stage-2 blocks double-buffer in the
                # 8-bank PSUM.  Row group g = tile parity; both tiles'
                # streams run concurrently.
                ts = (2 * ms, 2 * ms + 1)
                for p in (0, 1):
                    accs = {t: psum_pool.tile([128, 4, W], F32,
                                              name="ps", tag="ps")
                            for t in ts}
                    for k, (ky, kx) in enumerate(TAP_ORDER):
                        idx = (ky * 3 + kx) * 2 + p
                        for t in ts:
                            g = t % 2
                            gs = slice(g * C, (g + 1) * C)
                            r_lo = max(4 * t, 1 - ky)
                            r_hi = min(4 * t + 3, 128 - ky)
                            nr = r_hi - r_lo + 1
                            c_lo = max(0, 1 - kx)
                            x_lo = c_lo + kx - 1
                            ncc = 128 - abs(kx - 1)
                            rhs = x_s[gs, r_lo + ky - 1:r_lo + ky - 1 + nr,
                                      x_lo:x_lo + ncc]
                            dst = accs[t][:, r_lo - 4 * t:r_lo - 4 * t + nr,
                                          c_lo:c_lo + ncc]
                            mm(dst, w1_s[gs, idx * 128:(idx + 1) * 128],
                               rhs, k == 0, k == 8, (g * C, 0))
                    for t in ts:
                        dst = ybufs[p][0][:, 4 * t:4 * t + 4, :]
                        if t % 2 == 0:
                            nc.scalar.activation(dst, accs[t][:, :, :], IDENT,
                                                 bias=b1_s[:, p:p + 1])
                        else:
                            nc.vector.tensor_scalar_add(dst, accs[t][:, :, :],
                                                        b1_s[:, p:p + 1])
                    # duplicate rows 8ms..8ms+8 with partition halves
                    # swapped; Scalar/Vector DMA queues (Sync/GpSimd carry
                    # the bulk x-in/out traffic).
                    r0, r1 = 8 * ms, 8 * ms + 8
                    nc.scalar.dma_start(
                        out=ybufs[p][1][0:C, r0:r1, :],
                        in_=ybufs[p][0][C:128, r0:r1, :])
                    nc.scalar.dma_start(
                        out=ybufs[p][1][C:128, r0:r1, :],
                        in_=ybufs[p][0][0:C, r0:r1, :])

            def stage2_block(j):
                # Four accumulators shared by both spatial tiles (col
                # group c -> PSUM partitions [64c, 64c+64)); emission
                # round-robins across the four (q, c) streams so all
                # four sub-arrays stream concurrently.  Slot 2k/2k+1
                # share tap TAP_ORDER[k]; slots 0/1 are the interior
                # tap = the start-flag write for accs (0,q)/(1,q).
                st = stg.tile([128, 8, 2 * W], F32, name="st", tag="st")
                accs = {pq: psum_pool.tile([128, 4, W], F32,
                                           name="ps", tag="ps")
                        for pq in COMPS}
                for s in range(18):
                    ky, kx = TAP_ORDER[s // 2]
                    p = s % 2
                    ti = ky * 3 + kx
                    blk = ti * C
                    for q in (0, 1):
                        gs = slice(q * C, (q + 1) * C)
                        p_in, q_in, dy, dx = _tap_decomp(p, q, ky, kx)
                        for cg in (0, 1):
                            t = 2 * j + cg
                            cs = slice(cg * C, (cg + 1) * C)
                            r_lo = max(4 * t, -dy)
                            r_hi = min(4 * t + 3, 127 - dy)
                            nr = r_hi - r_lo + 1
                            c_lo = max(0, -dx)
                            y_lo = c_lo + dx
                            ncc = 128 - abs(dx)
                            rhs = ybufs[p_in][q_in ^ q][
                                gs, r_lo + dy:r_lo + dy + nr,
                                y_lo:y_lo + ncc]
                            dst = accs[p, q][cs,
                                             r_lo - 4 * t:r_lo - 4 * t + nr,
                                             c_lo:c_lo + ncc]
                            mm(dst, w2_s[gs, blk:blk + C], rhs,
                               s < 2, s >= 16, (q * C, cg * C))
                for i, (p, q) in enumerate(COMPS):
                    dst = st[:, p::2, q::2]
                    if i % 2 == 0:
                        nc.scalar.activation(dst, accs[p, q][:, :, :],
                                             IDENT, bias=b2_s[:, 0:1])
                    else:
                        nc.vector.tensor_scalar_add(
                            dst, accs[p, q][:, :, :], b2_s[:, 0:1])
                nc.sync.dma_start(out=out_d[:, 16 * j:16 * j + 8, :],
                                  in_=st[0:C, :, :])
                nc.gpsimd.dma_start(out=out_d[:, 16 * j + 8:16 * j + 16, :],
                                    in_=st[C:128, :, :])

            # ---- interleave: block j reads y rows through 8j+8, i.e.
            # mini-supers through j+1 ----
            stage1_ms(0)
            for j in range(H // 8):
                if j + 1 < H // 8:
                    stage1_ms(j + 1)
                stage2_block(j)

    nc.compile()
    return nc


_MODULE_CACHE = {}


def _get_module():
    if "nc" not in _MODULE_CACHE:
        _MODULE_CACHE["nc"] = _build_module()
    return _MODULE_CACHE["nc"]


# ----------------------------------------------------------------------------
# Entry point
# ----------------------------------------------------------------------------

def prep_weight_map(pre_w, pre_b, post_w, post_b):
    """Device-layout weight arrays, shared across cores."""
    w1, b1 = _build_stage1_weights(np.asarray(pre_w), np.asarray(pre_b))
    w2 = _build_stage2_weights(np.asarray(post_w))
    b2 = np.asarray(post_b, np.float32).reshape(C, 1)

    w1_half = np.transpose(w1, (3, 1, 2, 0, 4)).reshape(C, 18 * 128)
    # w1_half[cin, ((ky*3+kx)*2+p)*128 + m] = w1[p, ky, kx, cin, m]
    w1_flat = np.ascontiguousarray(
        np.concatenate([w1_half, w1_half], axis=0)).astype(NP_BF16)
    w2_flat = np.ascontiguousarray(w2).astype(NP_BF16)
    return {
        "w1": w1_flat,
        "b1": np.ascontiguousarray(b1, np.float32),                # [128, 2]
        "w2": w2_flat,
        "b2": np.ascontiguousarray(np.vstack([b2, b2]), np.float32),
    }


def run(x, pre_w, pre_b, post_w, post_b, trace=False):
    x = np.asarray(x, np.float32)
    B = x.shape[0]
    assert B == N_CORES and x.shape == (B, C, H, W)

    wmap = prep_weight_map(pre_w, pre_b, post_w, post_b)
    x_bf = x.astype(NP_BF16)

    in_maps = []
    for b in range(B):
        in_maps.append({"x": np.ascontiguousarray(x_bf[b]), **wmap})

    nc = _get_module()
    res = run_bass_kernel_spmd(nc, in_maps, core_ids=list(range(N_CORES)),
                               trace=trace)
    out = np.stack([res.results[b]["out"] for b in range(B)])
    return out, res


def kernel(x, pre_w, pre_b, post_w, post_b):
    out, _ = run(x, pre_w, pre_b, post_w, post_b)
    return out


# revision 8
# speedup vs baseline: 1.7750x; 1.3234x over previous
"""DiscreteWaveletUpsample Trainium2 kernel.

Math: out = conv3x3(haar_upsample(conv3x3(x, pre_w) + pre_b), post_w) + post_b

Device algorithm (per core, one batch sample, data-parallel over batch=8):

  * The fixed Haar reconstruction (stride-2 transposed conv with
    non-overlapping 2x2 taps) is folded into the pre-conv weights:
    Y(p,q)[c,h,w] (the (p,q) polyphase components of the upsampled image,
    y[c, 2h+p, 2w+q] = Y(p,q)[c,h,w]) is itself a 3x3 conv of x with
    effective weights  Weff[p,q,c] = sum_s haar[s,p,q] * pre_w[s*64+c].

  * All SBUF images are DENSE (no zero-pad halo).  Border taps emit
    narrower matmuls into row/col-offset PSUM sub-windows; the always-
    interior tap (ky=1,kx=1) goes first so its start-flag clears the
    whole accumulator.  Dense layout keeps every DMA run >= 4 KB
    (the padded layout's 256 B runs ran the input load at 62 GB/s).

  * Stage 1 (mini-super = 2 spatial 4-row tiles, per out ctile p): 9
    tap-matmuls with K=cin=64 accumulate [Y(p,0); Y(p,1)] (M=128) in
    PSUM.  The PE runs in 64x128 row-tiled mode: x is duplicated on both
    partition halves and the two tiles alternate row groups, so both
    matmul streams execute concurrently on the two sub-arrays.
    Evacuation (ScalarE/VectorE alternating) adds the bias and writes
    bf16 into dense SBUF images.

  * Stage 2 = the post conv in polyphase space: output component (p,q)
    at (h,w) sums 9 taps, each a K=64 matmul against component
    (p_in,q_in) at offset (dy,dx).  The PE runs in 64x64 four-tile
    mode: row group = INPUT component parity (q_in), column group =
    spatial-tile parity.  Keying row groups on q_in means every stream
    reads the stage-1 evac-natural ybuf layout (Y(p,0) on partitions
    0-63, Y(p,1) on 64-127) directly — no partition-swapped duplicate
    images, no SBUF-SBUF DMA (the dup DMAs stalled the PE ~2.4us per
    block waiting on the copy queue).  The four accumulators are SHARED
    between the two spatial tiles of a block (col group c writes PSUM
    partitions [64c, 64c+64)), so a block needs 4 PSUM banks and blocks
    double-buffer in the 8-bank PSUM.  Emission round-robins single
    matmuls across the four (q_in, c) streams so the in-order PE queue
    keeps all four sub-arrays streaming (tile-major emission capped
    concurrency at ~2.2 of 4).  An accumulator now receives matmuls
    from BOTH row-group streams; the slot schedule makes its start-flag
    matmul (interior tap, own-parity stream, slot 0/1) execute >=2
    slots before any other-stream contribution, and its stop-flag
    matmul (slot 16/17) >=2 slots after the other stream's last one,
    so the in-order dispatch plus equal-length streams guarantee
    clear-before-accumulate ordering.  Taps reading row 8j+8 (produced
    by the next mini-super) sit late in the schedule.
    Evacuation is full-width (both col groups at once) and interleaves
    components into full-resolution rows in SBUF staging.

  * Full-res rows DMA to HBM per block, alternating Sync/GpSimd queues;
    the x load rides Sync+GpSimd up front and weights ride Scalar, so
    no queue serializes compute-critical transfers behind bulk output.
"""

import os

import numpy as np
import ml_dtypes

import concourse.bass as bass
import concourse.mybir as mybir
import concourse.tile as tile
from concourse import bacc
from concourse.tile_rust import add_dep_helper
from concourse.bass_utils import run_bass_kernel_spmd

N_CORES = 8

C = 64          # channels (cin = cout = 64; stage-1 produces 4*C subbands)
H = W = 128     # input spatial dims
TAPS9 = [(ky, kx) for ky in range(3) for kx in range(3)]
# interior-full tap first: its start-flag write covers the whole acc
TAP_ORDER = [(1, 1)] + [t for t in TAPS9 if t != (1, 1)]
COMPS = [(0, 0), (0, 1), (1, 0), (1, 1)]

F32 = mybir.dt.float32
BF16 = mybir.dt.bfloat16
NP_BF16 = ml_dtypes.bfloat16

IDENT = mybir.ActivationFunctionType.Identity


# ----------------------------------------------------------------------------
# Host-side weight preparation
# ----------------------------------------------------------------------------

def _build_stage1_weights(pre_w, pre_b):
    """Fold the Haar reconstruction into the pre-conv weights.

    Returns
      w1[p, ky, kx, cin, m] float32, m = q*64 + c
      b1[m, p] float32
    """
    lo = np.array([0.5, 0.5], np.float32)
    hi = np.array([0.5, -0.5], np.float32)
    filt = np.stack([np.outer(lo, lo), np.outer(lo, hi),
                     np.outer(hi, lo), np.outer(hi, hi)], axis=0)  # [4,2,2]
    pw = pre_w.reshape(4, C, C, 3, 3).astype(np.float32)
    pb = pre_b.reshape(4, C).astype(np.float32)
    weff = np.einsum('spq,scikl->pqcikl', filt, pw)   # [p,q,c,cin,ky,kx]
    beff = np.einsum('spq,sc->pqc', filt, pb)         # [p,q,c]
    w1 = np.transpose(weff, (0, 4, 5, 3, 1, 2)).reshape(2, 3, 3, C, 2 * C)
    b1 = beff.reshape(2, 2 * C).T.copy()              # [m, p]
    return w1, b1


def _tap_decomp(p, q, ky, kx):
    """Polyphase decomposition of full-res tap (ky,kx) for out comp (p,q):
    returns (p_in, q_in, dy, dx)."""
    jy = p + ky - 1
    p_in = jy & 1
    dy = (jy - p_in) >> 1
    jx = q + kx - 1
    q_in = jx & 1
    dx = (jx - q_in) >> 1
    return p_in, q_in, dy, dx


def _build_stage2_weights(post_w):
    """w2[128, 9*64] bf16-ready float32.

    The stage-2 lhsT depends only on the tap: column block ti holds
    post_w[:, :, ky, kx].T [cin 64, cout 64], duplicated on both
    partition halves so either row group can load it."""
    w2 = np.zeros((2 * C, 9 * C), np.float32)
    pwf = post_w.astype(np.float32)
    for ti, (ky, kx) in enumerate(TAPS9):
        blk = ti * C
        w2[0:C, blk:blk + C] = pwf[:, :, ky, kx].T
        w2[C:2 * C, blk:blk + C] = pwf[:, :, ky, kx].T
    return w2


# ----------------------------------------------------------------------------
# Device module
# ----------------------------------------------------------------------------

def _build_module():
    nc = bacc.Bacc("TRN2", target_bir_lowering=False, debug=False,
                   num_devices=N_CORES)

    x_d = nc.dram_tensor("x", [C, H, W], BF16, kind="ExternalInput")
    w1_d = nc.dram_tensor("w1", [128, 18 * 128], BF16, kind="ExternalInput")
    b1_d = nc.dram_tensor("b1", [128, 2], F32, kind="ExternalInput")
    w2_d = nc.dram_tensor("w2", [128, 9 * C], BF16, kind="ExternalInput")
    b2_d = nc.dram_tensor("b2", [128, 1], F32, kind="ExternalInput")
    out_d = nc.dram_tensor("out", [C, 2 * H, 2 * W], F32,
                           kind="ExternalOutput")

    with tile.TileContext(nc) as tc:
        with (
            tc.tile_pool(name="const", bufs=1) as const,
            tc.tile_pool(name="xbuf", bufs=1) as xpool,
            tc.tile_pool(name="ybuf", bufs=1) as ypool,
            tc.tile_pool(name="psum", bufs=8, space="PSUM") as psum_pool,
            tc.tile_pool(name="stage", bufs=4) as stg,
        ):
            # ---- constants + input, spread over the three DMA queues so
            # the head transfers land in parallel: mini-super 0 waits on
            # x chunk 0 (both halves) + w1; b1 (Scalar) beats the first
            # evac.  w1's first-used block pair (interior tap, idx 8/9)
            # leads the split w1 loads.
            w1_s = const.tile([128, 18 * 128], BF16)
            w2_s = const.tile([128, 9 * C], BF16)
            b1_s = const.tile([128, 2], F32)
            nc.scalar.dma_start(out=b1_s[:], in_=b1_d[:])
            b2_s = const.tile([128, 1], F32)
            nc.scalar.dma_start(out=b2_s[:], in_=b2_d[:])
            nc.scalar.dma_start(out=w2_s[:], in_=w2_d[:])

            x_s = xpool.tile([128, H, W], BF16)
            nc.sync.dma_start(out=x_s[0:C, 0:32, :], in_=x_d[:, 0:32, :])
            nc.gpsimd.dma_start(out=x_s[C:128, 0:32, :], in_=x_d[:, 0:32, :])
            nc.sync.dma_start(out=w1_s[:, 8 * 128:10 * 128],
                              in_=w1_d[:, 8 * 128:10 * 128])
            nc.gpsimd.dma_start(out=w1_s[:, 0 * 128:8 * 128],
                                in_=w1_d[:, 0 * 128:8 * 128])
            nc.sync.dma_start(out=w1_s[:, 10 * 128:18 * 128],
                              in_=w1_d[:, 10 * 128:18 * 128])
            for r0 in range(32, H, 32):
                nc.sync.dma_start(out=x_s[0:C, r0:r0 + 32, :],
                                  in_=x_d[:, r0:r0 + 32, :])
                nc.gpsimd.dma_start(out=x_s[C:128, r0:r0 + 32, :],
                                    in_=x_d[:, r0:r0 + 32, :])

            # ---- Y buffers, dense ----
            # ybufs[p]: partitions 0-63 = Y(p,0), 64-127 = Y(p,1) — the
            # stage-1 evac-natural layout, read directly by stage 2.
            # Every element is evac-written before stage-2 reads it, so
            # no memsets are needed.
            ybufs = [ypool.tile([128, H, W], BF16, name=f"ybuf{p}")
                     for p in (0, 1)]

            # ---- matmul emission: global PE order chain ----
            state = {"prev": None}

            def mm(out_ap, w_ap, rhs_ap, start, stop, pos):
                inst = nc.tensor.matmul(out_ap, w_ap, rhs_ap,
                                        start=start, stop=stop,
                                        tile_position=pos)
                if state["prev"] is not None:
                    add_dep_helper(inst.ins, state["prev"], sync=False,
                                   reason="pe-emission-order")
                state["prev"] = inst.ins

            def stage1_ms(ms):
                # Mini-super of 2 spatial tiles: 4 PSUM banks, so
                # mini-supers and stage-2 blocks double-buffer in the
                # 8-bank PSUM.  Row group g = tile parity; both tiles'
                # streams run concurrently.
                ts = (2 * ms, 2 * ms + 1)
                for p in (0, 1):
                    accs = {t: psum_pool.tile([128, 4, W], F32,
                                              name="ps", tag="ps")
                            for t in ts}
                    for k, (ky, kx) in enumerate(TAP_ORDER):
                        idx = (ky * 3 + kx) * 2 + p
                        for t in ts:
                            g = t % 2
                            gs = slice(g * C, (g + 1) * C)
                            r_lo = max(4 * t, 1 - ky)
                            r_hi = min(4 * t + 3, 128 - ky)
                            nr = r_hi - r_lo + 1
                            c_lo = max(0, 1 - kx)
                            x_lo = c_lo + kx - 1
                            ncc = 128 - abs(kx - 1)
                            rhs = x_s[gs, r_lo + ky - 1:r_lo + ky - 1 + nr,
                                      x_lo:x_lo + ncc]
                            dst = accs[t][:, r_lo - 4 * t:r_lo - 4 * t + nr,
                                          c_lo:c_lo + ncc]
                            mm(dst, w1_s[gs, idx * 128:(idx + 1) * 128],
                               rhs, k == 0, k == 8, (g * C, 0))
                    for t in ts:
                        dst = ybufs[p][:, 4 * t:4 * t + 4, :]
                        if t % 2 == 0:
                            nc.scalar.activation(dst, accs[t][:, :, :], IDENT,
                                                 bias=b1_s[:, p:p + 1])
                        else:
                            nc.vector.tensor_scalar_add(dst, accs[t][:, :, :],
                                                        b1_s[:, p:p + 1])

            # Per-row-group stream schedules: stream g handles all taps
            # with q_in == g.  Slot layout (18 slots, p-pairs share a
            # tap so consecutive LDWEIGHTS repeat):
            #   0-1   : q=g tap (1,1)  — start-flag (interior, full acc)
            #   2-9   : q=1-g taps (ky in 0..1, kx in {0,2})
            #   10-11 : q=g tap (0,1)
            #   12-15 : q=1-g taps (ky=2, kx in {0,2})  — read row 8j+8
            #   16-17 : q=g tap (2,1)  — stop-flag; p=1 reads row 8j+8
            def _stream_sched(g):
                sched = [(0, g, 1, 1), (1, g, 1, 1)]
                sched += [(p, 1 - g, ky, kx)
                          for ky in (0, 1) for kx in (0, 2) for p in (0, 1)]
                sched += [(0, g, 0, 1), (1, g, 0, 1)]
                sched += [(p, 1 - g, 2, kx) for kx in (0, 2) for p in (0, 1)]
                sched += [(0, g, 2, 1), (1, g, 2, 1)]
                return sched

            SCHED = [_stream_sched(0), _stream_sched(1)]

            def stage2_block(j):
                # Four accumulators shared by both spatial tiles (col
                # group c -> PSUM partitions [64c, 64c+64)); emission
                # round-robins across the four (q_in, c) streams so all
                # four sub-arrays stream concurrently.
                st = stg.tile([128, 8, 2 * W], F32, name="st", tag="st")
                accs = {pq: psum_pool.tile([128, 4, W], F32,
                                           name="ps", tag="ps")
                        for pq in COMPS}
                for s in range(18):
                    for g in (0, 1):
                        p, q, ky, kx = SCHED[g][s]
                        p_in, q_in, dy, dx = _tap_decomp(p, q, ky, kx)
                        assert q_in == g
                        ti = ky * 3 + kx
                        blk = ti * C
                        gs = slice(g * C, (g + 1) * C)
                        for cg in (0, 1):
                            t = 2 * j + cg
                            cs = slice(cg * C, (cg + 1) * C)
                            r_lo = max(4 * t, -dy)
                            r_hi = min(4 * t + 3, 127 - dy)
                            nr = r_hi - r_lo + 1
                            c_lo = max(0, -dx)
                            y_lo = c_lo + dx
                            ncc = 128 - abs(dx)
                            rhs = ybufs[p_in][
                                gs, r_lo + dy:r_lo + dy + nr,
                                y_lo:y_lo + ncc]
                            dst = accs[p, q][cs,
                                             r_lo - 4 * t:r_lo - 4 * t + nr,
                                             c_lo:c_lo + ncc]
                            mm(dst, w2_s[gs, blk:blk + C], rhs,
                               s < 2, s >= 16, (g * C, cg * C))
                for i, (p, q) in enumerate(COMPS):
                    dst = st[:, p::2, q::2]
                    if i % 2 == 0:
                        nc.scalar.activation(dst, accs[p, q][:, :, :],
                                             IDENT, bias=b2_s[:, 0:1])
                    else:
                        nc.vector.tensor_scalar_add(
                            dst, accs[p, q][:, :, :], b2_s[:, 0:1])
                nc.sync.dma_start(out=out_d[:, 16 * j:16 * j + 8, :],
                                  in_=st[0:C, :, :])
                nc.gpsimd.dma_start(out=out_d[:, 16 * j + 8:16 * j + 16, :],
                                    in_=st[C:128, :, :])

            # ---- interleave: block j reads y rows through 8j+8, i.e.
            # mini-supers through j+1 ----
            stage1_ms(0)
            for j in range(H // 8):
                if j + 1 < H // 8:
                    stage1_ms(j + 1)
                stage2_block(j)

    nc.compile()
    return nc


_MODULE_CACHE = {}


def _get_module():
    if "nc" not in _MODULE_CACHE:
        _MODULE_CACHE["nc"] = _build_module()
    return _MODULE_CACHE["nc"]


# ----------------------------------------------------------------------------
# Entry point
# ----------------------------------------------------------------------------

def prep_weight_map(pre_w, pre_b, post_w, post_b):
    """Device-layout weight arrays, shared across cores."""
    w1, b1 = _build_stage1_weights(np.asarray(pre_w), np.asarray(pre_b))
    w2 = _build_stage2_weights(np.asarray(post_w))
    b2 = np.asarray(post_b, np.float32).reshape(C, 1)

    w1_half = np.transpose(w1, (3, 1, 2, 0, 4)).reshape(C, 18 * 128)
    # w1_half[cin, ((ky*3+kx)*2+p)*128 + m] = w1[p, ky, kx, cin, m]
    w1_flat = np.ascontiguousarray(
        np.concatenate([w1_half, w1_half], axis=0)).astype(NP_BF16)
    w2_flat = np.ascontiguousarray(w2).astype(NP_BF16)
    return {
        "w1": w1_flat,
        "b1": np.ascontiguousarray(b1, np.float32),                # [128, 2]
        "w2": w2_flat,
        "b2": np.ascontiguousarray(np.vstack([b2, b2]), np.float32),
    }


def run(x, pre_w, pre_b, post_w, post_b, trace=False):
    x = np.asarray(x, np.float32)
    B = x.shape[0]
    assert B == N_CORES and x.shape == (B, C, H, W)

    wmap = prep_weight_map(pre_w, pre_b, post_w, post_b)
    x_bf = x.astype(NP_BF16)

    in_maps = []
    for b in range(B):
        in_maps.append({"x": np.ascontiguousarray(x_bf[b]), **wmap})

    nc = _get_module()
    res = run_bass_kernel_spmd(nc, in_maps, core_ids=list(range(N_CORES)),
                               trace=trace)
    out = np.stack([res.results[b]["out"] for b in range(B)])
    return out, res


def kernel(x, pre_w, pre_b, post_w, post_b):
    out, _ = run(x, pre_w, pre_b, post_w, post_b)
    return out


# revision 23
# speedup vs baseline: 1.8134x; 1.0216x over previous
"""DiscreteWaveletUpsample Trainium2 kernel.

Math: out = conv3x3(haar_upsample(conv3x3(x, pre_w) + pre_b), post_w) + post_b

Device algorithm (per core, one batch sample, data-parallel over batch=8):

  * The fixed Haar reconstruction (stride-2 transposed conv with
    non-overlapping 2x2 taps) is folded into the pre-conv weights:
    Y(p,q)[c,h,w] (the (p,q) polyphase components of the upsampled image,
    y[c, 2h+p, 2w+q] = Y(p,q)[c,h,w]) is itself a 3x3 conv of x with
    effective weights  Weff[p,q,c] = sum_s haar[s,p,q] * pre_w[s*64+c].

  * All SBUF images are DENSE (no zero-pad halo).  Border taps emit
    narrower matmuls into row/col-offset PSUM sub-windows; the always-
    interior tap (ky=1,kx=1) goes first so its start-flag clears the
    whole accumulator.  Dense layout keeps every DMA run >= 4 KB
    (the padded layout's 256 B runs ran the input load at 62 GB/s).

  * Stage 1 (mini-super = 2 spatial 4-row tiles, per out ctile p): 9
    tap-matmuls with K=cin=64 accumulate [Y(p,0); Y(p,1)] (M=128) in
    PSUM.  The PE runs in 64x128 row-tiled mode: x is duplicated on
    both partition halves and the two tiles alternate row groups, so
    both matmul streams execute concurrently.  (The measured cost of
    alternating with stage 2's 64x64 mode is zero at the boundaries; a
    uniform-quad stage 1 hung the device — its two col-group streams
    would read the identical rhs AP.)  Evacuation (ScalarE/VectorE
    alternating) adds the bias and writes bf16 into dense SBUF images.

  * Stage 2 = the post conv in polyphase space: output component (p,q)
    at (h,w) sums 9 taps, each a K=64 matmul against component
    (p_in,q_in) at offset (dy,dx).  The PE runs in 64x64 four-tile
    mode: row group = INPUT component parity (q_in), column group =
    spatial-tile parity.  Keying row groups on q_in means every stream
    reads the stage-1 evac-natural ybuf layout (Y(p,0) on partitions
    0-63, Y(p,1) on 64-127) directly — no partition-swapped duplicate
    images, no SBUF-SBUF DMA (the dup DMAs stalled the PE ~2.4us per
    block waiting on the copy queue).  The four accumulators are SHARED
    between the two spatial tiles of a block (col group c writes PSUM
    partitions [64c, 64c+64)), so a block needs 4 PSUM banks and blocks
    double-buffer in the 8-bank PSUM.  Emission round-robins single
    matmuls across the four (q_in, c) streams so the in-order PE queue
    keeps all four sub-arrays streaming (tile-major emission capped
    concurrency at ~2.2 of 4).  An accumulator now receives matmuls
    from BOTH row-group streams; the slot schedule makes its start-flag
    matmul (interior tap, own-parity stream, slot 0/1) execute >=2
    slots before any other-stream contribution, and its stop-flag
    matmul (slot 16/17) >=2 slots after the other stream's last one,
    so the in-order dispatch plus equal-length streams guarantee
    clear-before-accumulate ordering.  Taps reading row 8j+8 (produced
    by the next mini-super) sit late in the schedule.
    Evacuation is full-width (both col groups at once) and interleaves
    components into full-resolution rows in SBUF staging.

  * Full-res rows DMA to HBM per block, alternating Sync/GpSimd queues;
    the x load rides Sync+GpSimd up front and weights ride Scalar, so
    no queue serializes compute-critical transfers behind bulk output.
"""

import os

import numpy as np
import ml_dtypes

import concourse.bass as bass
import concourse.mybir as mybir
import concourse.tile as tile
from concourse import bacc
from concourse.tile_rust import add_dep_helper
from concourse.bass_utils import run_bass_kernel_spmd

N_CORES = 8

C = 64          # channels (cin = cout = 64; stage-1 produces 4*C subbands)
H = W = 128     # input spatial dims
TAPS9 = [(ky, kx) for ky in range(3) for kx in range(3)]
# interior-full tap first: its start-flag write covers the whole acc
TAP_ORDER = [(1, 1)] + [t for t in TAPS9 if t != (1, 1)]
COMPS = [(0, 0), (0, 1), (1, 0), (1, 1)]

F32 = mybir.dt.float32
BF16 = mybir.dt.bfloat16
NP_BF16 = ml_dtypes.bfloat16

IDENT = mybir.ActivationFunctionType.Identity


# ----------------------------------------------------------------------------
# Host-side weight preparation
# ----------------------------------------------------------------------------

def _build_stage1_weights(pre_w, pre_b):
    """Fold the Haar reconstruction into the pre-conv weights.

    Returns
      w1[p, ky, kx, cin, m] float32, m = q*64 + c
      b1[m, p] float32
    """
    lo = np.array([0.5, 0.5], np.float32)
    hi = np.array([0.5, -0.5], np.float32)
    filt = np.stack([np.outer(lo, lo), np.outer(lo, hi),
                     np.outer(hi, lo), np.outer(hi, hi)], axis=0)  # [4,2,2]
    pw = pre_w.reshape(4, C, C, 3, 3).astype(np.float32)
    pb = pre_b.reshape(4, C).astype(np.float32)
    weff = np.einsum('spq,scikl->pqcikl', filt, pw)   # [p,q,c,cin,ky,kx]
    beff = np.einsum('spq,sc->pqc', filt, pb)         # [p,q,c]
    w1 = np.transpose(weff, (0, 4, 5, 3, 1, 2)).reshape(2, 3, 3, C, 2 * C)
    b1 = beff.reshape(2, 2 * C).T.copy()              # [m, p]
    return w1, b1


def _tap_decomp(p, q, ky, kx):
    """Polyphase decomposition of full-res tap (ky,kx) for out comp (p,q):
    returns (p_in, q_in, dy, dx)."""
    jy = p + ky - 1
    p_in = jy & 1
    dy = (jy - p_in) >> 1
    jx = q + kx - 1
    q_in = jx & 1
    dx = (jx - q_in) >> 1
    return p_in, q_in, dy, dx


def _build_stage2_weights(post_w):
    """w2[128, 9*64] bf16-ready float32.

    The stage-2 lhsT depends only on the tap: column block ti holds
    post_w[:, :, ky, kx].T [cin 64, cout 64], duplicated on both
    partition halves so either row group can load it."""
    w2 = np.zeros((2 * C, 9 * C), np.float32)
    pwf = post_w.astype(np.float32)
    for ti, (ky, kx) in enumerate(TAPS9):
        blk = ti * C
        w2[0:C, blk:blk + C] = pwf[:, :, ky, kx].T
        w2[C:2 * C, blk:blk + C] = pwf[:, :, ky, kx].T
    return w2


# ----------------------------------------------------------------------------
# Device module
# ----------------------------------------------------------------------------

def _build_module():
    nc = bacc.Bacc("TRN2", target_bir_lowering=False, debug=False,
                   num_devices=N_CORES)

    x_d = nc.dram_tensor("x", [C, H, W], BF16, kind="ExternalInput")
    w1_d = nc.dram_tensor("w1", [128, 18 * 128], BF16, kind="ExternalInput")
    b1_d = nc.dram_tensor("b1", [128, 2], F32, kind="ExternalInput")
    w2_d = nc.dram_tensor("w2", [128, 9 * C], BF16, kind="ExternalInput")
    b2_d = nc.dram_tensor("b2", [128, 1], F32, kind="ExternalInput")
    out_d = nc.dram_tensor("out", [C, 2 * H, 2 * W], F32,
                           kind="ExternalOutput")

    with tile.TileContext(nc) as tc:
        with (
            tc.tile_pool(name="const", bufs=1) as const,
            tc.tile_pool(name="xbuf", bufs=1) as xpool,
            tc.tile_pool(name="ybuf", bufs=1) as ypool,
            tc.tile_pool(name="psum", bufs=8, space="PSUM") as psum_pool,
            tc.tile_pool(name="stage", bufs=4) as stg,
        ):
            # ---- constants + input, spread over the three DMA queues so
            # the head transfers land in parallel: mini-super 0 waits on
            # x chunk 0 (both halves) + w1; b1 (Scalar) beats the first
            # evac.  w1's first-used block pair (interior tap, idx 8/9)
            # leads the split w1 loads.
            w1_s = const.tile([128, 18 * 128], BF16)
            w2_s = const.tile([128, 9 * C], BF16)
            b1_s = const.tile([128, 2], F32)
            nc.scalar.dma_start(out=b1_s[:], in_=b1_d[:])
            b2_s = const.tile([128, 1], F32)

            # w1 block idx = (ky*3+kx)*2 + p: ms0's p=0 phase consumes the
            # even blocks in TAP_ORDER sequence (8, 0, 2, 4, 6, 10, ...),
            # p=1 the odds one phase later.  The first matmul gates on
            # w1[8] + x chunk 0; the idle-at-head Scalar queue delivers
            # blocks 0-7 in small pieces before the ramp consumes them,
            # GpSimd prefetches the tail blocks behind its x chunk.
            def w1_load(eng, lo, hi):
                eng.dma_start(out=w1_s[:, lo * 128:hi * 128],
                              in_=w1_d[:, lo * 128:hi * 128])

            x_s = xpool.tile([128, H, W], BF16)
            w1_load(nc.sync, 8, 9)
            nc.sync.dma_start(out=x_s[0:C, 0:16, :], in_=x_d[:, 0:16, :])
            w1_load(nc.gpsimd, 9, 10)
            nc.gpsimd.dma_start(out=x_s[C:128, 0:16, :], in_=x_d[:, 0:16, :])
            for lo in (0, 2, 4, 6):
                w1_load(nc.scalar, lo, lo + 2)
            nc.scalar.dma_start(out=w2_s[:], in_=w2_d[:])
            nc.scalar.dma_start(out=b2_s[:], in_=b2_d[:])
            w1_load(nc.gpsimd, 10, 14)
            w1_load(nc.gpsimd, 14, 18)
            nc.sync.dma_start(out=x_s[0:C, 16:32, :], in_=x_d[:, 16:32, :])
            nc.gpsimd.dma_start(out=x_s[C:128, 16:32, :],
                                in_=x_d[:, 16:32, :])
            for r0 in range(32, H, 32):
                nc.sync.dma_start(out=x_s[0:C, r0:r0 + 32, :],
                                  in_=x_d[:, r0:r0 + 32, :])
                nc.gpsimd.dma_start(out=x_s[C:128, r0:r0 + 32, :],
                                    in_=x_d[:, r0:r0 + 32, :])

            # ---- Y buffers, dense ----
            # ybufs[p]: partitions 0-63 = Y(p,0), 64-127 = Y(p,1) — the
            # stage-1 evac-natural layout, read directly by stage 2.
            # Every element is evac-written before stage-2 reads it, so
            # no memsets are needed.
            ybufs = [ypool.tile([128, H, W], BF16, name=f"ybuf{p}")
                     for p in (0, 1)]

            # ---- matmul emission: global PE order chain ----
            state = {"prev": None}

            def mm(out_ap, w_ap, rhs_ap, start, stop, pos):
                inst = nc.tensor.matmul(out_ap, w_ap, rhs_ap,
                                        start=start, stop=stop,
                                        tile_position=pos)
                if state["prev"] is not None:
                    add_dep_helper(inst.ins, state["prev"], sync=False,
                                   reason="pe-emission-order")
                state["prev"] = inst.ins

            def stage1_ms(ms):
                # Mini-super of 2 spatial tiles: 4 PSUM banks, so
                # mini-supers and stage-2 blocks double-buffer in the
                # 8-bank PSUM.  Same 64x64 quad mode as stage 2: row
                # group = tile parity (over the duplicated-x halves),
                # col group = q-half of the 128 output channels; the
                # four streams run concurrently and each tile's two
                # col-group streams share its PSUM bank.
                ts = (2 * ms, 2 * ms + 1)
                for p in (0, 1):
                    accs = {t: psum_pool.tile([128, 4, W], F32,
                                              name="ps", tag="ps")
                            for t in ts}
                    for k, (ky, kx) in enumerate(TAP_ORDER):
                        idx = (ky * 3 + kx) * 2 + p
                        for t in ts:
                            g = t % 2
                            gs = slice(g * C, (g + 1) * C)
                            r_lo = max(4 * t, 1 - ky)
                            r_hi = min(4 * t + 3, 128 - ky)
                            nr = r_hi - r_lo + 1
                            c_lo = max(0, 1 - kx)
                            x_lo = c_lo + kx - 1
                            ncc = 128 - abs(kx - 1)
                            rhs = x_s[gs, r_lo + ky - 1:r_lo + ky - 1 + nr,
                                      x_lo:x_lo + ncc]
                            dst = accs[t][:, r_lo - 4 * t:r_lo - 4 * t + nr,
                                          c_lo:c_lo + ncc]
                            mm(dst, w1_s[gs, idx * 128:(idx + 1) * 128],
                               rhs, k == 0, k == 8, (g * C, 0))
                    for t in ts:
                        dst = ybufs[p][:, 4 * t:4 * t + 4, :]
                        if t % 2 == 0:
                            nc.scalar.activation(dst, accs[t][:, :, :], IDENT,
                                                 bias=b1_s[:, p:p + 1])
                        else:
                            nc.vector.tensor_scalar_add(dst, accs[t][:, :, :],
                                                        b1_s[:, p:p + 1])

            # Per-row-group stream schedules: stream g handles all taps
            # with q_in == g.  Slot layout (18 slots, p-pairs share a
            # tap so consecutive LDWEIGHTS repeat):
            #   0-1   : q=g tap (1,1)  — start-flag (interior, full acc)
            #   2-9   : q=1-g taps (ky in 0..1, kx in {0,2})
            #   10-11 : q=g tap (0,1)
            #   12-15 : q=1-g taps (ky=2, kx in {0,2})  — read row 8j+8
            #   16-17 : q=g tap (2,1)  — stop-flag; p=1 reads row 8j+8
            def _stream_sched(g):
                sched = [(0, g, 1, 1), (1, g, 1, 1)]
                sched += [(p, 1 - g, ky, kx)
                          for ky in (0, 1) for kx in (0, 2) for p in (0, 1)]
                sched += [(0, g, 0, 1), (1, g, 0, 1)]
                sched += [(p, 1 - g, 2, kx) for kx in (0, 2) for p in (0, 1)]
                sched += [(0, g, 2, 1), (1, g, 2, 1)]
                return sched

            SCHED = [_stream_sched(0), _stream_sched(1)]
            out_q = [nc.sync, nc.gpsimd, nc.scalar]

            def stage2_block(j):
                # Four accumulators shared by both spatial tiles (col
                # group c -> PSUM partitions [64c, 64c+64)); emission
                # round-robins across the four (q_in, c) streams so all
                # four sub-arrays stream concurrently.
                st = stg.tile([128, 8, 2 * W], F32, name="st", tag="st")
                accs = {pq: psum_pool.tile([128, 4, W], F32,
                                           name="ps", tag="ps")
                        for pq in COMPS}
                for s in range(18):
                    for g in (0, 1):
                        p, q, ky, kx = SCHED[g][s]
                        p_in, q_in, dy, dx = _tap_decomp(p, q, ky, kx)
                        assert q_in == g
                        ti = ky * 3 + kx
                        blk = ti * C
                        gs = slice(g * C, (g + 1) * C)
                        for cg in (0, 1):
                            t = 2 * j + cg
                            cs = slice(cg * C, (cg + 1) * C)
                            r_lo = max(4 * t, -dy)
                            r_hi = min(4 * t + 3, 127 - dy)
                            nr = r_hi - r_lo + 1
                            c_lo = max(0, -dx)
                            y_lo = c_lo + dx
                            ncc = 128 - abs(dx)
                            rhs = ybufs[p_in][
                                gs, r_lo + dy:r_lo + dy + nr,
                                y_lo:y_lo + ncc]
                            dst = accs[p, q][cs,
                                             r_lo - 4 * t:r_lo - 4 * t + nr,
                                             c_lo:c_lo + ncc]
                            mm(dst, w2_s[gs, blk:blk + C], rhs,
                               s < 2, s >= 16, (g * C, cg * C))
                for i, (p, q) in enumerate(COMPS):
                    dst = st[:, p::2, q::2]
                    if i % 2 == 0:
                        nc.scalar.activation(dst, accs[p, q][:, :, :],
                                             IDENT, bias=b2_s[:, 0:1])
                    else:
                        nc.vector.tensor_scalar_add(
                            dst, accs[p, q][:, :, :], b2_s[:, 0:1])
                # 4x 256KB pieces rotating over all three DMA queues:
                # balances the 16MB output across queues and keeps the
                # final block's drain (after the last matmul) short.
                for i in range(4):
                    half, rr = i // 2, 4 * (i % 2)
                    out_q[(4 * j + i) % 3].dma_start(
                        out=out_d[:, 16 * j + 8 * half + rr:
                                  16 * j + 8 * half + rr + 4, :],
                        in_=st[half * C:(half + 1) * C, rr:rr + 4, :])

            # ---- interleave: block j reads y rows through 8j+8, i.e.
            # mini-supers through j+1 ----
            stage1_ms(0)
            for j in range(H // 8):
                if j + 1 < H // 8:
                    stage1_ms(j + 1)
                stage2_block(j)

    nc.compile()
    return nc


_MODULE_CACHE = {}


def _get_module():
    if "nc" not in _MODULE_CACHE:
        _MODULE_CACHE["nc"] = _build_module()
    return _MODULE_CACHE["nc"]


# ----------------------------------------------------------------------------
# Entry point
# ----------------------------------------------------------------------------

def prep_weight_map(pre_w, pre_b, post_w, post_b):
    """Device-layout weight arrays, shared across cores."""
    w1, b1 = _build_stage1_weights(np.asarray(pre_w), np.asarray(pre_b))
    w2 = _build_stage2_weights(np.asarray(post_w))
    b2 = np.asarray(post_b, np.float32).reshape(C, 1)

    w1_half = np.transpose(w1, (3, 1, 2, 0, 4)).reshape(C, 18 * 128)
    # w1_half[cin, ((ky*3+kx)*2+p)*128 + m] = w1[p, ky, kx, cin, m]
    w1_flat = np.ascontiguousarray(
        np.concatenate([w1_half, w1_half], axis=0)).astype(NP_BF16)
    w2_flat = np.ascontiguousarray(w2).astype(NP_BF16)
    return {
        "w1": w1_flat,
        "b1": np.ascontiguousarray(b1, np.float32),                # [128, 2]
        "w2": w2_flat,
        "b2": np.ascontiguousarray(np.vstack([b2, b2]), np.float32),
    }


def run(x, pre_w, pre_b, post_w, post_b, trace=False):
    x = np.asarray(x, np.float32)
    B = x.shape[0]
    assert B == N_CORES and x.shape == (B, C, H, W)

    wmap = prep_weight_map(pre_w, pre_b, post_w, post_b)
    x_bf = x.astype(NP_BF16)

    in_maps = []
    for b in range(B):
        in_maps.append({"x": np.ascontiguousarray(x_bf[b]), **wmap})

    nc = _get_module()
    res = run_bass_kernel_spmd(nc, in_maps, core_ids=list(range(N_CORES)),
                               trace=trace)
    out = np.stack([res.results[b]["out"] for b in range(B)])
    return out, res


def kernel(x, pre_w, pre_b, post_w, post_b):
    out, _ = run(x, pre_w, pre_b, post_w, post_b)
    return out


# revision 28
# speedup vs baseline: 1.8380x; 1.0136x over previous
"""DiscreteWaveletUpsample Trainium2 kernel.

Math: out = conv3x3(haar_upsample(conv3x3(x, pre_w) + pre_b), post_w) + post_b

Device algorithm (per core, one batch sample, data-parallel over batch=8):

  * The fixed Haar reconstruction (stride-2 transposed conv with
    non-overlapping 2x2 taps) is folded into the pre-conv weights:
    Y(p,q)[c,h,w] (the (p,q) polyphase components of the upsampled image,
    y[c, 2h+p, 2w+q] = Y(p,q)[c,h,w]) is itself a 3x3 conv of x with
    effective weights  Weff[p,q,c] = sum_s haar[s,p,q] * pre_w[s*64+c].

  * All SBUF images are DENSE (no zero-pad halo).  Border taps emit
    narrower matmuls into row/col-offset PSUM sub-windows; the always-
    interior tap (ky=1,kx=1) goes first so its start-flag clears the
    whole accumulator.  Dense layout keeps every DMA run >= 4 KB
    (the padded layout's 256 B runs ran the input load at 62 GB/s).

  * Stage 1 (mini-super = 2 spatial 4-row tiles, per out ctile p): 9
    tap-matmuls with K=cin=64 accumulate [Y(p,0); Y(p,1)] (M=128) in
    PSUM.  The PE runs in 64x128 row-tiled mode: x is duplicated on
    both partition halves and the two tiles alternate row groups, so
    both matmul streams execute concurrently.  (The measured cost of
    alternating with stage 2's 64x64 mode is zero at the boundaries; a
    uniform-quad stage 1 hung the device — its two col-group streams
    would read the identical rhs AP.)  Evacuation (ScalarE/VectorE
    alternating) adds the bias and writes bf16 into dense SBUF images.

  * Stage 2 = the post conv in polyphase space: output component (p,q)
    at (h,w) sums 9 taps, each a K=64 matmul against component
    (p_in,q_in) at offset (dy,dx).  The PE runs in 64x64 four-tile
    mode: row group = INPUT component parity (q_in), column group =
    spatial-tile parity.  Keying row groups on q_in means every stream
    reads the stage-1 evac-natural ybuf layout (Y(p,0) on partitions
    0-63, Y(p,1) on 64-127) directly — no partition-swapped duplicate
    images, no SBUF-SBUF DMA (the dup DMAs stalled the PE ~2.4us per
    block waiting on the copy queue).  The four accumulators are SHARED
    between the two spatial tiles of a block (col group c writes PSUM
    partitions [64c, 64c+64)), so a block needs 4 PSUM banks and blocks
    double-buffer in the 8-bank PSUM.  Emission round-robins single
    matmuls across the four (q_in, c) streams so the in-order PE queue
    keeps all four sub-arrays streaming (tile-major emission capped
    concurrency at ~2.2 of 4).  An accumulator now receives matmuls
    from BOTH row-group streams; the slot schedule makes its start-flag
    matmul (interior tap, own-parity stream, slot 0/1) execute >=2
    slots before any other-stream contribution, and its stop-flag
    matmul (slot 16/17) >=2 slots after the other stream's last one,
    so the in-order dispatch plus equal-length streams guarantee
    clear-before-accumulate ordering.  Taps reading row 8j+8 (produced
    by the next mini-super) sit late in the schedule.
    Evacuation is full-width (both col groups at once) and interleaves
    components into full-resolution rows in SBUF staging.

  * Full-res rows DMA to HBM per block, alternating Sync/GpSimd queues;
    the x load rides Sync+GpSimd up front and weights ride Scalar, so
    no queue serializes compute-critical transfers behind bulk output.
"""

import os

import numpy as np
import ml_dtypes

import concourse.bass as bass
import concourse.mybir as mybir
import concourse.tile as tile
from concourse import bacc
from concourse.tile_rust import add_dep_helper
from concourse.bass_utils import run_bass_kernel_spmd

N_CORES = 8

C = 64          # channels (cin = cout = 64; stage-1 produces 4*C subbands)
H = W = 128     # input spatial dims
TAPS9 = [(ky, kx) for ky in range(3) for kx in range(3)]
# interior-full tap first: its start-flag write covers the whole acc
TAP_ORDER = [(1, 1)] + [t for t in TAPS9 if t != (1, 1)]
COMPS = [(0, 0), (0, 1), (1, 0), (1, 1)]

# w1 column blocks in device consumption order (ms0: p=0 phase consumes
# its 9 taps' blocks, then p=1), so each head load group is one
# contiguous dram slice with >=1KB runs.  W1POS[idx] = position of
# original block idx = (ky*3+kx)*2 + p.
_W1SEQ = ([(ky * 3 + kx) * 2 for ky, kx in TAP_ORDER]
          + [(ky * 3 + kx) * 2 + 1 for ky, kx in TAP_ORDER])
W1POS = {idx: pos for pos, idx in enumerate(_W1SEQ)}

F32 = mybir.dt.float32
BF16 = mybir.dt.bfloat16
NP_BF16 = ml_dtypes.bfloat16

IDENT = mybir.ActivationFunctionType.Identity


# ----------------------------------------------------------------------------
# Host-side weight preparation
# ----------------------------------------------------------------------------

def _build_stage1_weights(pre_w, pre_b):
    """Fold the Haar reconstruction into the pre-conv weights.

    Returns
      w1[p, ky, kx, cin, m] float32, m = q*64 + c
      b1[m, p] float32
    """
    lo = np.array([0.5, 0.5], np.float32)
    hi = np.array([0.5, -0.5], np.float32)
    filt = np.stack([np.outer(lo, lo), np.outer(lo, hi),
                     np.outer(hi, lo), np.outer(hi, hi)], axis=0)  # [4,2,2]
    pw = pre_w.reshape(4, C, C, 3, 3).astype(np.float32)
    pb = pre_b.reshape(4, C).astype(np.float32)
    weff = np.einsum('spq,scikl->pqcikl', filt, pw)   # [p,q,c,cin,ky,kx]
    beff = np.einsum('spq,sc->pqc', filt, pb)         # [p,q,c]
    w1 = np.transpose(weff, (0, 4, 5, 3, 1, 2)).reshape(2, 3, 3, C, 2 * C)
    b1 = beff.reshape(2, 2 * C).T.copy()              # [m, p]
    return w1, b1


def _tap_decomp(p, q, ky, kx):
    """Polyphase decomposition of full-res tap (ky,kx) for out comp (p,q):
    returns (p_in, q_in, dy, dx)."""
    jy = p + ky - 1
    p_in = jy & 1
    dy = (jy - p_in) >> 1
    jx = q + kx - 1
    q_in = jx & 1
    dx = (jx - q_in) >> 1
    return p_in, q_in, dy, dx


def _build_stage2_weights(post_w):
    """w2[128, 9*64] bf16-ready float32.

    The stage-2 lhsT depends only on the tap: column block ti holds
    post_w[:, :, ky, kx].T [cin 64, cout 64], duplicated on both
    partition halves so either row group can load it."""
    w2 = np.zeros((2 * C, 9 * C), np.float32)
    pwf = post_w.astype(np.float32)
    for ti, (ky, kx) in enumerate(TAPS9):
        blk = ti * C
        w2[0:C, blk:blk + C] = pwf[:, :, ky, kx].T
        w2[C:2 * C, blk:blk + C] = pwf[:, :, ky, kx].T
    return w2


# ----------------------------------------------------------------------------
# Device module
# ----------------------------------------------------------------------------

def _build_module():
    nc = bacc.Bacc("TRN2", target_bir_lowering=False, debug=False,
                   num_devices=N_CORES)

    x_d = nc.dram_tensor("x", [C, H, W], BF16, kind="ExternalInput")
    w1_d = nc.dram_tensor("w1", [128, 18 * 128], BF16, kind="ExternalInput")
    b1_d = nc.dram_tensor("b1", [128, 2], F32, kind="ExternalInput")
    w2_d = nc.dram_tensor("w2", [128, 9 * C], BF16, kind="ExternalInput")
    b2_d = nc.dram_tensor("b2", [128, 1], F32, kind="ExternalInput")
    out_d = nc.dram_tensor("out", [C, 2 * H, 2 * W], F32,
                           kind="ExternalOutput")

    with tile.TileContext(nc) as tc:
        with (
            tc.tile_pool(name="const", bufs=1) as const,
            tc.tile_pool(name="xbuf", bufs=1) as xpool,
            tc.tile_pool(name="ybuf", bufs=1) as ypool,
            tc.tile_pool(name="psum", bufs=8, space="PSUM") as psum_pool,
            tc.tile_pool(name="stage", bufs=4) as stg,
        ):
            # ---- constants + input, spread over the three DMA queues so
            # the head transfers land in parallel: mini-super 0 waits on
            # x chunk 0 (both halves) + w1; b1 (Scalar) beats the first
            # evac.  w1's first-used block pair (interior tap, idx 8/9)
            # leads the split w1 loads.
            w1_s = const.tile([128, 18 * 128], BF16)
            w2_s = const.tile([128, 9 * C], BF16)
            b1_s = const.tile([128, 2], F32)
            nc.scalar.dma_start(out=b1_s[:], in_=b1_d[:])
            b2_s = const.tile([128, 1], F32)

            # w1 lives in consumption-order positions (W1POS): position
            # ranges are contiguous dram slices.  The first matmul gates
            # on pos 0 (the interior tap's p=0 block) + x rows 0-11;
            # each queue then delivers the next-needed group just ahead
            # of the ramp: Sync feeds p0-early behind its x chunk,
            # Scalar (idle at head) p0-late + p1-late, GpSimd p1-early.
            def w1_load(eng, lo, hi):
                eng.dma_start(out=w1_s[:, lo * 128:hi * 128],
                              in_=w1_d[:, lo * 128:hi * 128])

            x_s = xpool.tile([128, H, W], BF16)
            w1_load(nc.sync, 0, 1)                                 # pos 0
            nc.sync.dma_start(out=x_s[0:C, 0:12, :], in_=x_d[:, 0:12, :])
            w1_load(nc.gpsimd, 9, 10)                              # p1 first
            nc.gpsimd.dma_start(out=x_s[C:128, 0:12, :], in_=x_d[:, 0:12, :])
            w1_load(nc.sync, 1, 5)                                 # p0 early
            w1_load(nc.scalar, 5, 9)                               # p0 late
            w1_load(nc.gpsimd, 10, 14)                             # p1 early
            w1_load(nc.scalar, 14, 18)                             # p1 late
            nc.scalar.dma_start(out=w2_s[:], in_=w2_d[:])
            nc.scalar.dma_start(out=b2_s[:], in_=b2_d[:])
            nc.sync.dma_start(out=x_s[0:C, 12:24, :], in_=x_d[:, 12:24, :])
            nc.gpsimd.dma_start(out=x_s[C:128, 12:24, :],
                                in_=x_d[:, 12:24, :])
            nc.sync.dma_start(out=x_s[0:C, 24:32, :], in_=x_d[:, 24:32, :])
            nc.gpsimd.dma_start(out=x_s[C:128, 24:32, :],
                                in_=x_d[:, 24:32, :])
            for r0 in range(32, H, 32):
                nc.sync.dma_start(out=x_s[0:C, r0:r0 + 32, :],
                                  in_=x_d[:, r0:r0 + 32, :])
                nc.gpsimd.dma_start(out=x_s[C:128, r0:r0 + 32, :],
                                    in_=x_d[:, r0:r0 + 32, :])

            # ---- Y buffers, dense ----
            # ybufs[p]: partitions 0-63 = Y(p,0), 64-127 = Y(p,1) — the
            # stage-1 evac-natural layout, read directly by stage 2.
            # Every element is evac-written before stage-2 reads it, so
            # no memsets are needed.
            ybufs = [ypool.tile([128, H, W], BF16, name=f"ybuf{p}")
                     for p in (0, 1)]

            # ---- matmul emission: global PE order chain ----
            state = {"prev": None}

            def mm(out_ap, w_ap, rhs_ap, start, stop, pos):
                inst = nc.tensor.matmul(out_ap, w_ap, rhs_ap,
                                        start=start, stop=stop,
                                        tile_position=pos)
                if state["prev"] is not None:
                    add_dep_helper(inst.ins, state["prev"], sync=False,
                                   reason="pe-emission-order")
                state["prev"] = inst.ins

            def stage1_ms(ms):
                # Mini-super of 2 spatial tiles: 4 PSUM banks, so
                # mini-supers and stage-2 blocks double-buffer in the
                # 8-bank PSUM.  Same 64x64 quad mode as stage 2: row
                # group = tile parity (over the duplicated-x halves),
                # col group = q-half of the 128 output channels; the
                # four streams run concurrently and each tile's two
                # col-group streams share its PSUM bank.
                ts = (2 * ms, 2 * ms + 1)
                for p in (0, 1):
                    accs = {t: psum_pool.tile([128, 4, W], F32,
                                              name="ps", tag="ps")
                            for t in ts}
                    for k, (ky, kx) in enumerate(TAP_ORDER):
                        idx = W1POS[(ky * 3 + kx) * 2 + p]
                        for t in ts:
                            g = t % 2
                            gs = slice(g * C, (g + 1) * C)
                            r_lo = max(4 * t, 1 - ky)
                            r_hi = min(4 * t + 3, 128 - ky)
                            nr = r_hi - r_lo + 1
                            c_lo = max(0, 1 - kx)
                            x_lo = c_lo + kx - 1
                            ncc = 128 - abs(kx - 1)
                            rhs = x_s[gs, r_lo + ky - 1:r_lo + ky - 1 + nr,
                                      x_lo:x_lo + ncc]
                            dst = accs[t][:, r_lo - 4 * t:r_lo - 4 * t + nr,
                                          c_lo:c_lo + ncc]
                            mm(dst, w1_s[gs, idx * 128:(idx + 1) * 128],
                               rhs, k == 0, k == 8, (g * C, 0))
                    for t in ts:
                        dst = ybufs[p][:, 4 * t:4 * t + 4, :]
                        if t % 2 == 0:
                            nc.scalar.activation(dst, accs[t][:, :, :], IDENT,
                                                 bias=b1_s[:, p:p + 1])
                        else:
                            nc.vector.tensor_scalar_add(dst, accs[t][:, :, :],
                                                        b1_s[:, p:p + 1])

            # Per-row-group stream schedules: stream g handles all taps
            # with q_in == g.  Slot layout (18 slots, p-pairs share a
            # tap so consecutive LDWEIGHTS repeat):
            #   0-1   : q=g tap (1,1)  — start-flag (interior, full acc)
            #   2-9   : q=1-g taps (ky in 0..1, kx in {0,2})
            #   10-11 : q=g tap (0,1)
            #   12-15 : q=1-g taps (ky=2, kx in {0,2})  — read row 8j+8
            #   16-17 : q=g tap (2,1)  — stop-flag; p=1 reads row 8j+8
            def _stream_sched(g):
                sched = [(0, g, 1, 1), (1, g, 1, 1)]
                sched += [(p, 1 - g, ky, kx)
                          for ky in (0, 1) for kx in (0, 2) for p in (0, 1)]
                sched += [(0, g, 0, 1), (1, g, 0, 1)]
                sched += [(p, 1 - g, 2, kx) for kx in (0, 2) for p in (0, 1)]
                sched += [(0, g, 2, 1), (1, g, 2, 1)]
                return sched

            SCHED = [_stream_sched(0), _stream_sched(1)]
            out_q = [nc.sync, nc.gpsimd, nc.scalar]

            def stage2_block(j):
                # Four accumulators shared by both spatial tiles (col
                # group c -> PSUM partitions [64c, 64c+64)); emission
                # round-robins across the four (q_in, c) streams so all
                # four sub-arrays stream concurrently.
                st = stg.tile([128, 8, 2 * W], F32, name="st", tag="st")
                accs = {pq: psum_pool.tile([128, 4, W], F32,
                                           name="ps", tag="ps")
                        for pq in COMPS}
                for s in range(18):
                    for g in (0, 1):
                        p, q, ky, kx = SCHED[g][s]
                        p_in, q_in, dy, dx = _tap_decomp(p, q, ky, kx)
                        assert q_in == g
                        ti = ky * 3 + kx
                        blk = ti * C
                        gs = slice(g * C, (g + 1) * C)
                        for cg in (0, 1):
                            t = 2 * j + cg
                            cs = slice(cg * C, (cg + 1) * C)
                            r_lo = max(4 * t, -dy)
                            r_hi = min(4 * t + 3, 127 - dy)
                            nr = r_hi - r_lo + 1
                            c_lo = max(0, -dx)
                            y_lo = c_lo + dx
                            ncc = 128 - abs(dx)
                            rhs = ybufs[p_in][
                                gs, r_lo + dy:r_lo + dy + nr,
                                y_lo:y_lo + ncc]
                            dst = accs[p, q][cs,
                                             r_lo - 4 * t:r_lo - 4 * t + nr,
                                             c_lo:c_lo + ncc]
                            mm(dst, w2_s[gs, blk:blk + C], rhs,
                               s < 2, s >= 16, (g * C, cg * C))
                for i, (p, q) in enumerate(COMPS):
                    dst = st[:, p::2, q::2]
                    if i % 2 == 0:
                        nc.scalar.activation(dst, accs[p, q][:, :, :],
                                             IDENT, bias=b2_s[:, 0:1])
                    else:
                        nc.vector.tensor_scalar_add(
                            dst, accs[p, q][:, :, :], b2_s[:, 0:1])
                # 256KB pieces rotating over all three DMA queues:
                # balances the 16MB output across queues; the last two
                # blocks use 128KB pieces so the post-compute drain
                # (after the final matmul) stays short.
                np_ = 4 if j < 14 else 8
                rows = 8 // (np_ // 2)
                for i in range(np_):
                    half, rr = i // (np_ // 2), rows * (i % (np_ // 2))
                    out_q[(np_ * j + i) % 3].dma_start(
                        out=out_d[:, 16 * j + 8 * half + rr:
                                  16 * j + 8 * half + rr + rows, :],
                        in_=st[half * C:(half + 1) * C, rr:rr + rows, :])

            # ---- interleave: block j reads y rows through 8j+8, i.e.
            # mini-supers through j+1 ----
            stage1_ms(0)
            for j in range(H // 8):
                if j + 1 < H // 8:
                    stage1_ms(j + 1)
                stage2_block(j)

    nc.compile()
    return nc


_MODULE_CACHE = {}


def _get_module():
    if "nc" not in _MODULE_CACHE:
        _MODULE_CACHE["nc"] = _build_module()
    return _MODULE_CACHE["nc"]


# ----------------------------------------------------------------------------
# Entry point
# ----------------------------------------------------------------------------

def prep_weight_map(pre_w, pre_b, post_w, post_b):
    """Device-layout weight arrays, shared across cores."""
    w1, b1 = _build_stage1_weights(np.asarray(pre_w), np.asarray(pre_b))
    w2 = _build_stage2_weights(np.asarray(post_w))
    b2 = np.asarray(post_b, np.float32).reshape(C, 1)

    w1_half = np.transpose(w1, (3, 1, 2, 0, 4)).reshape(C, 18 * 128)
    # w1_half[cin, ((ky*3+kx)*2+p)*128 + m] = w1[p, ky, kx, cin, m];
    # permute blocks into device consumption order (see W1POS)
    w1_half = w1_half.reshape(C, 18, 128)[:, _W1SEQ, :].reshape(C, 18 * 128)
    w1_flat = np.ascontiguousarray(
        np.concatenate([w1_half, w1_half], axis=0)).astype(NP_BF16)
    w2_flat = np.ascontiguousarray(w2).astype(NP_BF16)
    return {
        "w1": w1_flat,
        "b1": np.ascontiguousarray(b1, np.float32),                # [128, 2]
        "w2": w2_flat,
        "b2": np.ascontiguousarray(np.vstack([b2, b2]), np.float32),
    }


def run(x, pre_w, pre_b, post_w, post_b, trace=False):
    x = np.asarray(x, np.float32)
    B = x.shape[0]
    assert B == N_CORES and x.shape == (B, C, H, W)

    wmap = prep_weight_map(pre_w, pre_b, post_w, post_b)
    x_bf = x.astype(NP_BF16)

    in_maps = []
    for b in range(B):
        in_maps.append({"x": np.ascontiguousarray(x_bf[b]), **wmap})

    nc = _get_module()
    res = run_bass_kernel_spmd(nc, in_maps, core_ids=list(range(N_CORES)),
                               trace=trace)
    out = np.stack([res.results[b]["out"] for b in range(B)])
    return out, res


def kernel(x, pre_w, pre_b, post_w, post_b):
    out, _ = run(x, pre_w, pre_b, post_w, post_b)
    return out
